# revision 9
# baseline (speedup 1.0000x reference)
"""Trainium2 Bass kernel for GNN message-passing Coulomb potential.

reference math:
    pot = 1/r per edge; y[i] += pot*c[j]; y[j] += pot*c[i]; y *= 0.5

Strategy (edge/data parallel, owner-computes on destination):
  * Host-side sharding prep: expand each edge into its two (dst, src, r)
    contributions, group contributions by destination atom, order atoms by
    degree, and pack everything into a per-core [128, W] fp32 stream of
    fixed-shape blocks (8 cores x identical block schedule -> one SPMD NEFF).
    Each block holds JS_PER_BLOCK js x 8 cores atom-groups padded to a
    uniform per-atom slot count K_b.  Stream layout per core/partition/block
    (channel-major so the w broadcast is a rank-3 stride-0 AP):
        [ 2*r : (g,k) G*K | charges[src] : (ch,g,k) 4*G*K ]
    Padding slots carry r=0.5 (w=2) and charge 0 so they contribute 0.
  * Device (per core): stream blocks; w = reciprocal(2r) = 0.5/r on DVE;
    one fused custom-DVE pass computes the running sum of cg*w in place;
    per-(ch,group) segment sums are differences of the strided prefix ends.
    Cores own disjoint atom ranges -> no collective.
  * Host: invert the atom permutation to produce y [n_atoms, 4].
"""

import os
import sys

if "/opt/trn_rl_repo" not in sys.path:
    sys.path.insert(0, "/opt/trn_rl_repo")

import numpy as np

N_CORES = 8
JS_PER_BLOCK = 4  # js (per-core groups) batched into one DVE block
SEGS = 4 * JS_PER_BLOCK  # (ch, g) segments per block
GROUPS_PER_BLOCK = N_CORES * JS_PER_BLOCK
KMIN = 16
OUT_CHUNKS = 4

_MUL_PSCAN = None


def _get_mul_pscan_op():
    """Register (once) a custom DVE op: out = running sum of in0*in1."""
    global _MUL_PSCAN
    if _MUL_PSCAN is not None:
        return _MUL_PSCAN
    import concourse.dve_ops as dve_ops
    from concourse.dve_spec import (
        AluOp,
        Spec,
        Src0,
        Src1,
        _has_src1 as has_src1,
        lower,
        scan,
    )
    from concourse.dve_uop import DveOpSpec

    name = "MUL_PREFIX_SUM_ANT"
    for op in dve_ops.OPS:
        if op.name == name:
            _MUL_PSCAN = op
            return op

    def _ref(in0, in1, s0, s1, imm2):
        a = np.asarray(in0, np.float32).reshape(in0.shape[0], -1)
        b = np.asarray(in1, np.float32).reshape(in1.shape[0], -1)
        prod = (a * b).astype(np.float32)
        return np.cumsum(prod, axis=-1, dtype=np.float32)

    spec = Spec(body=scan(AluOp.ADD, Src0 * Src1), reference=_ref)
    opcode = max(dve_ops._SUB_OPCODE_FOR_NAME.values()) + 1
    assert opcode < 0x20
    uops_sha = {}
    for ver in ("v3", "v4"):
        try:
            tmp = DveOpSpec(
                name=name, opcode=opcode, uops=lower(spec, ver=ver),
                rd1_en=has_src1(spec),
            )
            uops_sha[ver] = tmp.sha(ver)
        except Exception:
            pass
    op = dve_ops.DveOp(name, spec, subdim=False, uops_sha=uops_sha)
    dve_ops.OPS.append(op)
    dve_ops.CUSTOM_DVE_SPECS[name] = spec
    dve_ops._SUB_OPCODE_FOR_NAME[name] = opcode
    _MUL_PSCAN = op
    return op


def _plan(deg):
    """Degree-descending atom ordering and uniform-K block schedule."""
    A = deg.shape[0]
    pi = np.argsort(-deg, kind="stable")  # atom ids, degree desc
    rank_of_atom = np.empty(A, np.int64)
    rank_of_atom[pi] = np.arange(A)

    ng_raw = -(-A // 128)  # ceil
    NG_TOT = -(-ng_raw // GROUPS_PER_BLOCK) * GROUPS_PER_BLOCK
    NATOM_PAD = NG_TOT * 128
    NB = NG_TOT // GROUPS_PER_BLOCK
    NJ = NG_TOT // N_CORES

    deg_sorted = np.zeros(NATOM_PAD, np.int64)
    deg_sorted[:A] = deg[pi]
    # degrees are non-increasing -> block max = first atom of the block
    Kb = deg_sorted[np.arange(NB) * GROUPS_PER_BLOCK * 128]
    Kb = np.maximum(Kb, KMIN)
    Kb = ((Kb + 3) // 4) * 4
    SW = np.zeros(NB + 1, np.int64)
    SW[1:] = np.cumsum(5 * JS_PER_BLOCK * Kb)  # block width = 5*G*K
    W = int(SW[-1])
    return pi, rank_of_atom, NB, NJ, Kb, SW, W


def _preprocess(charges, neighbor_indices, neighbor_distances):
    """Build per-core device streams + unpermute metadata."""
    A = charges.shape[0]
    G = JS_PER_BLOCK
    src = np.concatenate([neighbor_indices[:, 1], neighbor_indices[:, 0]]).astype(
        np.int64
    )
    dst = np.concatenate([neighbor_indices[:, 0], neighbor_indices[:, 1]]).astype(
        np.int64
    )
    rr = np.concatenate([neighbor_distances, neighbor_distances]).astype(np.float32)
    M = dst.shape[0]

    deg = np.bincount(dst, minlength=A)
    pi, rank_of_atom, NB, NJ, Kb, SW, W = _plan(deg)

    # within-atom slot index k for every contribution
    order = np.argsort(dst, kind="stable")
    starts = np.zeros(A + 1, np.int64)
    starts[1:] = np.cumsum(deg)
    k = np.empty(M, np.int64)
    k[order] = np.arange(M) - starts[dst[order]]

    r = rank_of_atom[dst]
    g = r >> 7  # // 128
    p = r & 127
    c = g & 7  # core
    j = g >> 3
    b = j // G  # block
    t = j - b * G  # j position within block

    Kb_t = Kb[b]
    base = SW[b]
    col_r = base + t * Kb_t + k
    # channel-major cg region: (ch, g, k)
    col_cg0 = base + G * Kb_t + t * Kb_t + k

    arr = np.zeros((N_CORES, 128, W), np.float32)
    # r-region padding must be a safe reciprocal input: fill with 0.5 -> w=2,
    # times cg=0 -> contribution 0.  (Real slots overwritten below.)
    rmask = np.zeros(W, bool)
    for bb in range(NB):
        rmask[SW[bb] : SW[bb] + G * Kb[bb]] = True
    arr[:, :, rmask] = 0.5

    flat = arr.reshape(-1)
    row = (c * 128 + p) * W
    flat[row + col_r] = 2.0 * rr  # device: w = recip(2r) = 0.5/r
    ch_charges = charges.astype(np.float32)
    for ch in range(4):
        flat[row + col_cg0 + ch * G * Kb_t] = ch_charges[src, ch]

    return arr, pi, NB, NJ, Kb, SW, W


_KERNEL_CACHE = {}


def _build_kernel(NB, NJ, Kb, SW, W):
    key = (NB, NJ, tuple(int(x) for x in Kb), W)
    if key in _KERNEL_CACHE:
        return _KERNEL_CACHE[key]

    import concourse.bacc as bacc
    import concourse.mybir as mybir
    from concourse.tile import TileContext

    pscan = _get_mul_pscan_op()
    G = JS_PER_BLOCK

    f32 = mybir.dt.float32
    nc = bacc.Bacc("TRN2", target_bir_lowering=False, debug=False, num_devices=N_CORES)
    stream = nc.dram_tensor("stream", [128, W], f32, kind="ExternalInput")
    out = nc.dram_tensor("out", [128, NJ * 4], f32, kind="ExternalOutput")

    # output chunk boundaries (in blocks) for early writeback
    chunk_edges = sorted({round(i * NB / OUT_CHUNKS) for i in range(OUT_CHUNKS + 1)})

    with TileContext(nc) as tc:
        with (
            tc.tile_pool(name="io", bufs=8) as iop,
            tc.tile_pool(name="wk", bufs=4) as wkp,
            tc.tile_pool(name="ob", bufs=1) as obp,
        ):
            ob = obp.tile([128, NJ * 4], f32)
            for b in range(NB):
                K = int(Kb[b])
                base = int(SW[b])
                t = iop.tile([128, 5 * G * K], f32, tag="in")
                nc.sync.dma_start(t[:, :], stream[:, base : base + 5 * G * K])
                w = wkp.tile([128, G * K], f32, tag="w")
                nc.vector.reciprocal_approx_fast(out=w[:, :], in_=t[:, 0 : G * K])
                # fused cg*w running prefix sum, in place over the cg region
                cg = t[:, G * K : 5 * G * K].rearrange("p (c n) -> p c n", c=4)
                wb = w[:, :].unsqueeze(1).broadcast_to([128, 4, G * K])
                nc.vector._custom_dve(pscan, out=cg, in0=cg, in1=wb)
                # segment sums: ends[m] - ends[m-1]; ends[m]=prefix[m*K+K-1]
                ends = t[:, G * K : 5 * G * K].rearrange(
                    "p (m k) -> p m k", k=K
                )[:, :, K - 1]
                oc = b * SEGS
                nc.scalar.copy(ob[:, oc : oc + 1], ends[:, 0:1])
                nc.vector.tensor_sub(
                    ob[:, oc + 1 : oc + SEGS], ends[:, 1:SEGS], ends[:, 0 : SEGS - 1]
                )
                # early writeback of completed output chunks
                for ci in range(len(chunk_edges) - 1):
                    if b == chunk_edges[ci + 1] - 1:
                        lo = chunk_edges[ci] * SEGS
                        hi = chunk_edges[ci + 1] * SEGS
                        nc.scalar.dma_start(out[:, lo:hi], ob[:, lo:hi])

    nc.compile()
    _KERNEL_CACHE[key] = nc
    return nc


def _postprocess(outs, pi, A, NJ):
    """outs: list of 8 [128, NJ*4] arrays -> y [A, 4].

    Output column layout per block b: col = SEGS*b + G*ch + t, t = j%G."""
    G = JS_PER_BLOCK
    O = np.stack(outs)  # [8, 128, NJ*4]
    ranks = np.arange(A)
    g = ranks >> 7
    p = ranks & 127
    c = g & 7
    j = g >> 3
    b = j // G
    t = j - b * G
    col0 = SEGS * b + t
    y = np.empty((A, 4), np.float32)
    for ch in range(4):
        y[pi, ch] = O[c, p, col0 + G * ch]
    return y


def kernel(charges, cell, positions, neighbor_indices, neighbor_distances):
    charges = np.asarray(charges, dtype=np.float32)
    neighbor_indices = np.asarray(neighbor_indices)
    neighbor_distances = np.asarray(neighbor_distances, dtype=np.float32)
    A = charges.shape[0]

    arr, pi, NB, NJ, Kb, SW, W = _preprocess(
        charges, neighbor_indices, neighbor_distances
    )
    nc = _build_kernel(NB, NJ, Kb, SW, W)

    from concourse.bass_utils import run_bass_kernel_spmd

    trace = bool(int(os.environ.get("KERNEL_TRACE", "0")))
    res = run_bass_kernel_spmd(
        nc,
        [{"stream": arr[ci]} for ci in range(N_CORES)],
        core_ids=list(range(N_CORES)),
        trace=trace,
    )
    if trace:
        kernel.last_exec_time_ns = res.exec_time_ns
        kernel.last_results = res
    outs = [res.results[ci]["out"] for ci in range(N_CORES)]
    return _postprocess(outs, pi, A, NJ)


def _emulate_device(arr, NB, NJ, Kb, SW):
    """Numpy emulation of the device kernel (for logic validation)."""
    G = JS_PER_BLOCK
    outs = []
    for ci in range(N_CORES):
        ob = np.zeros((128, NJ * 4), np.float32)
        for b in range(NB):
            K = int(Kb[b])
            base = int(SW[b])
            t = arr[ci][:, base : base + 5 * G * K]
            w = 1.0 / t[:, 0 : G * K]
            cg = t[:, G * K : 5 * G * K].reshape(128, 4, G, K)
            v = cg * w.reshape(128, 1, G, K)
            ob[:, b * SEGS : (b + 1) * SEGS] = v.sum(-1).reshape(128, SEGS)
        outs.append(ob)
    return outs


# revision 10
# speedup vs baseline: 1.1857x; 1.1857x over previous
"""Trainium2 Bass kernel for GNN message-passing Coulomb potential.

reference math:
    pot = 1/r per edge; y[i] += pot*c[j]; y[j] += pot*c[i]; y *= 0.5

Strategy (edge/data parallel, owner-computes on destination):
  * Host-side sharding prep: expand each edge into its two (dst, src, r)
    contributions, group contributions by destination atom, order atoms by
    degree, and pack everything into a per-core [128, W] fp32 stream of
    fixed-shape blocks (8 cores x identical block schedule -> one SPMD NEFF).
    Each block holds JS_PER_BLOCK js x 8 cores atom-groups padded to a
    uniform per-atom slot count K_b.  Stream layout per core/partition/block
    (channel-major so the w broadcast is a rank-3 stride-0 AP):
        [ 2*r : (g,k) G*K | charges[src] : (ch,g,k) 4*G*K ]
    Padding slots carry r=0.5 (w=2) and charge 0 so they contribute 0.
  * Device (per core): stream blocks; w = reciprocal(2r) = 0.5/r on DVE;
    one fused custom-DVE pass computes the running sum of cg*w in place;
    per-(ch,group) segment sums are differences of the strided prefix ends.
    Cores own disjoint atom ranges -> no collective.
  * Host: invert the atom permutation to produce y [n_atoms, 4].
"""

import os
import sys

if "/opt/trn_rl_repo" not in sys.path:
    sys.path.insert(0, "/opt/trn_rl_repo")

import numpy as np

N_CORES = 8
JS_PER_BLOCK = 4  # js (per-core groups) batched into one DVE block
SEGS = 4 * JS_PER_BLOCK  # (ch, g) segments per block
GROUPS_PER_BLOCK = N_CORES * JS_PER_BLOCK
KMIN = 16
OUT_CHUNKS = 4

_MUL_PSCAN = None


def _get_mul_pscan_op():
    """Register (once) a custom DVE op: out = running sum of in0*in1."""
    global _MUL_PSCAN
    if _MUL_PSCAN is not None:
        return _MUL_PSCAN
    import concourse.dve_ops as dve_ops
    from concourse.dve_spec import (
        AluOp,
        Spec,
        Src0,
        Src1,
        _has_src1 as has_src1,
        lower,
        scan,
    )
    from concourse.dve_uop import DveOpSpec

    name = "MUL_PREFIX_SUM_ANT"
    for op in dve_ops.OPS:
        if op.name == name:
            _MUL_PSCAN = op
            return op

    def _ref(in0, in1, s0, s1, imm2):
        a = np.asarray(in0, np.float32).reshape(in0.shape[0], -1)
        b = np.asarray(in1, np.float32).reshape(in1.shape[0], -1)
        prod = (a * b).astype(np.float32)
        return np.cumsum(prod, axis=-1, dtype=np.float32)

    spec = Spec(body=scan(AluOp.ADD, Src0 * Src1), reference=_ref)
    opcode = max(dve_ops._SUB_OPCODE_FOR_NAME.values()) + 1
    assert opcode < 0x20
    uops_sha = {}
    for ver in ("v3", "v4"):
        try:
            tmp = DveOpSpec(
                name=name, opcode=opcode, uops=lower(spec, ver=ver),
                rd1_en=has_src1(spec),
            )
            uops_sha[ver] = tmp.sha(ver)
        except Exception:
            pass
    op = dve_ops.DveOp(name, spec, subdim=False, uops_sha=uops_sha)
    dve_ops.OPS.append(op)
    dve_ops.CUSTOM_DVE_SPECS[name] = spec
    dve_ops._SUB_OPCODE_FOR_NAME[name] = opcode
    _MUL_PSCAN = op
    return op


def _plan(deg):
    """Degree-descending atom ordering and uniform-K block schedule."""
    A = deg.shape[0]
    pi = np.argsort(-deg, kind="stable")  # atom ids, degree desc
    rank_of_atom = np.empty(A, np.int64)
    rank_of_atom[pi] = np.arange(A)

    ng_raw = -(-A // 128)  # ceil
    NG_TOT = -(-ng_raw // GROUPS_PER_BLOCK) * GROUPS_PER_BLOCK
    NATOM_PAD = NG_TOT * 128
    NB = NG_TOT // GROUPS_PER_BLOCK
    NJ = NG_TOT // N_CORES

    deg_sorted = np.zeros(NATOM_PAD, np.int64)
    deg_sorted[:A] = deg[pi]
    # degrees are non-increasing -> block max = first atom of the block
    Kb = deg_sorted[np.arange(NB) * GROUPS_PER_BLOCK * 128]
    Kb = np.maximum(Kb, KMIN)
    Kb = ((Kb + 3) // 4) * 4
    SW = np.zeros(NB + 1, np.int64)
    SW[1:] = np.cumsum(5 * JS_PER_BLOCK * Kb)  # block width = 5*G*K
    W = int(SW[-1])
    return pi, rank_of_atom, NB, NJ, Kb, SW, W


def _preprocess(charges, neighbor_indices, neighbor_distances):
    """Build per-core device streams + unpermute metadata."""
    A = charges.shape[0]
    G = JS_PER_BLOCK
    src = np.concatenate([neighbor_indices[:, 1], neighbor_indices[:, 0]]).astype(
        np.int64
    )
    dst = np.concatenate([neighbor_indices[:, 0], neighbor_indices[:, 1]]).astype(
        np.int64
    )
    rr = np.concatenate([neighbor_distances, neighbor_distances]).astype(np.float32)
    M = dst.shape[0]

    deg = np.bincount(dst, minlength=A)
    pi, rank_of_atom, NB, NJ, Kb, SW, W = _plan(deg)

    # within-atom slot index k for every contribution
    order = np.argsort(dst, kind="stable")
    starts = np.zeros(A + 1, np.int64)
    starts[1:] = np.cumsum(deg)
    k = np.empty(M, np.int64)
    k[order] = np.arange(M) - starts[dst[order]]

    r = rank_of_atom[dst]
    g = r >> 7  # // 128
    p = r & 127
    c = g & 7  # core
    j = g >> 3
    b = j // G  # block
    t = j - b * G  # j position within block

    Kb_t = Kb[b]
    base = SW[b]
    col_r = base + t * Kb_t + k
    # channel-major cg region: (ch, g, k)
    col_cg0 = base + G * Kb_t + t * Kb_t + k

    arr = np.zeros((N_CORES, 128, W), np.float32)
    # r-region padding must be a safe reciprocal input: fill with 0.5 -> w=2,
    # times cg=0 -> contribution 0.  (Real slots overwritten below.)
    rmask = np.zeros(W, bool)
    for bb in range(NB):
        rmask[SW[bb] : SW[bb] + G * Kb[bb]] = True
    arr[:, :, rmask] = 0.5

    flat = arr.reshape(-1)
    row = (c * 128 + p) * W
    flat[row + col_r] = 2.0 * rr  # device: w = recip(2r) = 0.5/r
    ch_charges = charges.astype(np.float32)
    for ch in range(4):
        flat[row + col_cg0 + ch * G * Kb_t] = ch_charges[src, ch]

    return arr, pi, NB, NJ, Kb, SW, W


_KERNEL_CACHE = {}


def _build_kernel(NB, NJ, Kb, SW, W):
    key = (NB, NJ, tuple(int(x) for x in Kb), W)
    if key in _KERNEL_CACHE:
        return _KERNEL_CACHE[key]

    import concourse.bacc as bacc
    import concourse.mybir as mybir
    from concourse.tile import TileContext

    pscan = _get_mul_pscan_op()
    G = JS_PER_BLOCK

    f32 = mybir.dt.float32
    nc = bacc.Bacc("TRN2", target_bir_lowering=False, debug=False, num_devices=N_CORES)
    stream = nc.dram_tensor("stream", [128, W], f32, kind="ExternalInput")
    out = nc.dram_tensor("out", [128, NJ * 4], f32, kind="ExternalOutput")

    # output chunk boundaries (in blocks) for early writeback
    chunk_edges = sorted({round(i * NB / OUT_CHUNKS) for i in range(OUT_CHUNKS + 1)})

    with TileContext(nc) as tc:
        with (
            tc.tile_pool(name="io", bufs=8) as iop,
            tc.tile_pool(name="wk", bufs=4) as wkp,
            tc.tile_pool(name="ob", bufs=1) as obp,
        ):
            ob = obp.tile([128, NJ * 4], f32)
            for b in range(NB):
                K = int(Kb[b])
                base = int(SW[b])
                t = iop.tile([128, 5 * G * K], f32, tag="in")
                dma_eng = nc.sync if b % 2 == 0 else nc.scalar
                dma_eng.dma_start(t[:, :], stream[:, base : base + 5 * G * K])
                w = wkp.tile([128, G * K], f32, tag="w")
                nc.vector.reciprocal_approx_fast(out=w[:, :], in_=t[:, 0 : G * K])
                # fused cg*w running prefix sum, in place over the cg region
                cg = t[:, G * K : 5 * G * K].rearrange("p (c n) -> p c n", c=4)
                wb = w[:, :].unsqueeze(1).broadcast_to([128, 4, G * K])
                nc.vector._custom_dve(pscan, out=cg, in0=cg, in1=wb)
                # segment sums: ends[m] - ends[m-1]; ends[m]=prefix[m*K+K-1]
                ends = t[:, G * K : 5 * G * K].rearrange(
                    "p (m k) -> p m k", k=K
                )[:, :, K - 1]
                oc = b * SEGS
                nc.scalar.copy(ob[:, oc : oc + 1], ends[:, 0:1])
                nc.vector.tensor_sub(
                    ob[:, oc + 1 : oc + SEGS], ends[:, 1:SEGS], ends[:, 0 : SEGS - 1]
                )
                # early writeback of completed output chunks
                for ci in range(len(chunk_edges) - 1):
                    if b == chunk_edges[ci + 1] - 1:
                        lo = chunk_edges[ci] * SEGS
                        hi = chunk_edges[ci + 1] * SEGS
                        nc.scalar.dma_start(out[:, lo:hi], ob[:, lo:hi])

    nc.compile()
    _KERNEL_CACHE[key] = nc
    return nc


def _postprocess(outs, pi, A, NJ):
    """outs: list of 8 [128, NJ*4] arrays -> y [A, 4].

    Output column layout per block b: col = SEGS*b + G*ch + t, t = j%G."""
    G = JS_PER_BLOCK
    O = np.stack(outs)  # [8, 128, NJ*4]
    ranks = np.arange(A)
    g = ranks >> 7
    p = ranks & 127
    c = g & 7
    j = g >> 3
    b = j // G
    t = j - b * G
    col0 = SEGS * b + t
    y = np.empty((A, 4), np.float32)
    for ch in range(4):
        y[pi, ch] = O[c, p, col0 + G * ch]
    return y


def kernel(charges, cell, positions, neighbor_indices, neighbor_distances):
    charges = np.asarray(charges, dtype=np.float32)
    neighbor_indices = np.asarray(neighbor_indices)
    neighbor_distances = np.asarray(neighbor_distances, dtype=np.float32)
    A = charges.shape[0]

    arr, pi, NB, NJ, Kb, SW, W = _preprocess(
        charges, neighbor_indices, neighbor_distances
    )
    nc = _build_kernel(NB, NJ, Kb, SW, W)

    from concourse.bass_utils import run_bass_kernel_spmd

    trace = bool(int(os.environ.get("KERNEL_TRACE", "0")))
    res = run_bass_kernel_spmd(
        nc,
        [{"stream": arr[ci]} for ci in range(N_CORES)],
        core_ids=list(range(N_CORES)),
        trace=trace,
    )
    if trace:
        kernel.last_exec_time_ns = res.exec_time_ns
        kernel.last_results = res
    outs = [res.results[ci]["out"] for ci in range(N_CORES)]
    return _postprocess(outs, pi, A, NJ)


def _emulate_device(arr, NB, NJ, Kb, SW):
    """Numpy emulation of the device kernel (for logic validation)."""
    G = JS_PER_BLOCK
    outs = []
    for ci in range(N_CORES):
        ob = np.zeros((128, NJ * 4), np.float32)
        for b in range(NB):
            K = int(Kb[b])
            base = int(SW[b])
            t = arr[ci][:, base : base + 5 * G * K]
            w = 1.0 / t[:, 0 : G * K]
            cg = t[:, G * K : 5 * G * K].reshape(128, 4, G, K)
            v = cg * w.reshape(128, 1, G, K)
            ob[:, b * SEGS : (b + 1) * SEGS] = v.sum(-1).reshape(128, SEGS)
        outs.append(ob)
    return outs
